# revision 52
# baseline (speedup 1.0000x reference)
"""DeepseekV2-style MoE (64 experts, top-6 grouped sigmoid routing) on 8 TRN2
NeuronCores — expert-parallel, on-chip routing table.

Structure (HW exec ~0.9 ms/core, amortized ~1.25 ms/call):
- bf16 weights everywhere (expert + shared MLPs), bf16 dispatched
  activations (xhat/xtb); Silu fused on the Act engine. Routing numerics
  fp32; index/one-hot table matmuls in fp16 (ints <= 2048 exact; bf16
  corrupts ids > 256).
- Shared experts emitted at seams inside the routing emission so their
  PE/Act work overlaps routing's DVE chain.
- Per-expert rank via triangular ones/triu matmuls (PE), then the compact
  per-slot (token, weight) table is built ON-CHIP: is_equal one-hot over
  rank x iota, contracted with a (token_id, weight) fp16 payload on the
  PE; one small DRAM roundtrip rearranges it into the wrapped idx layout
  dma_gather/dma_scatter_add require. (The previous HBM payload
  scatter+readback cost ~200 us serial and ~20k static DMA descriptors,
  which also inflated per-call dispatch by ~0.6 ms.)
- 256-row gather granules pipelined with expert compute; 512-wide expert
  MLP blocks; weights streamed distance-2 on the SP HWDGE ring (moving
  wd loads to the ACT ring measured slower).
- out_part + single ReduceScatter in bf16 (chunked RS and mesh
  AllToAll+local-reduce both measured slower), final convert to fp32.
"""

import sys

sys.path.insert(0, "/opt/trn_rl_repo")

import numpy as np

from concourse import bacc, tile
import concourse.mybir as mybir
from concourse.bass_utils import run_bass_kernel_spmd

FP32 = mybir.dt.float32
FP32R = mybir.dt.float32r
BF16 = mybir.dt.bfloat16
F16 = mybir.dt.float16
I16 = mybir.dt.int16
ALU = mybir.AluOpType
AXL = mybir.AxisListType
ACTF = mybir.ActivationFunctionType

T = 2048
HID = 1024
E = 64
INTER = 704
K = 6
NG = 8
TG = 4
ISH = 1408
SCALE = 2.5
NC = 8
EPC = 8
NEG = -1.0e30

NTT = T // 128            # 16 token tiles
NH = HID // 128           # 8 hid chunks
IC_SIZES = [128, 128, 128, 128, 128, 64]
IC_OFFS = [0, 128, 256, 384, 512, 640]
GS = E // NG              # 8 experts / group
PAY_S = K * NTT           # 96
PAYW = 64                 # slot-scatter payload width (256B row stride min)

OUT_DT = BF16             # out_part + ReduceScatter dtype (BF16 or FP32)
RS_CH = 1                 # ReduceScatter chunks (chunking measured slower)
COMB = "rs"              # "rs": ncfw ReduceScatter; "a2a": mesh AllToAll
                          # + on-chip 8-shard reduce


def _host_routing_counts(x, gate_w, score_bias):
    logits = x.astype(np.float64) @ gate_w.T.astype(np.float64)
    scores = 1.0 / (1.0 + np.exp(-logits))
    sc = scores + score_bias[None, :]
    gs = sc.reshape(T, NG, GS)
    top2 = np.sort(gs, axis=-1)[:, :, -2:].sum(-1)
    gidx = np.argsort(-top2, axis=-1)[:, :TG]
    gmask = np.zeros((T, NG), np.float64)
    np.put_along_axis(gmask, gidx, 1.0, axis=1)
    smask = np.repeat(gmask, GS, axis=1)
    masked = np.where(smask > 0, sc, -np.inf)
    ids = np.argsort(-masked, axis=-1)[:, :K]
    cnt = np.zeros(E, np.int64)
    for k in range(K):
        cnt += np.bincount(ids[:, k], minlength=E)
    return cnt


def _placement(caps):
    """Rank-based packing: position p holds experts ranked [NC*p, NC*(p+1))."""
    order = np.argsort(-caps)
    bins = [[int(order[NC * p + c]) for p in range(EPC)] for c in range(NC)]
    cap_sched = [int(caps[order[NC * p]]) for p in range(EPC)]
    return bins, cap_sched


def _blocks(cap):
    sizes = [512] * (cap // 512)
    if cap % 512:
        sizes.append(cap % 512)
    return sizes


def build_graph(cap_sched, s_core):
    # 2x SWDGE descriptor ring (32KB carveout): one 512-row scatter is
    # 1024 descriptors and exactly fills the default ring, so every
    # launch DRAIN-stalled the gpsimd engine 4-24us waiting for space
    nc = bacc.Bacc("TRN2", target_bir_lowering=False, debug=False,
                   num_devices=NC, num_swdge_queues=2,
                   dynamic_dma_scratch_size=24576)

    x_d = nc.dram_tensor("x", [T, HID], FP32, kind="ExternalInput")
    gwT_d = nc.dram_tensor("gwT", [NH, 128, E], FP32, kind="ExternalInput")
    bias_d = nc.dram_tensor("bias_b", [128, E], FP32, kind="ExternalInput")
    wg_d = nc.dram_tensor("wg", [EPC, HID, INTER], BF16, kind="ExternalInput")
    wu_d = nc.dram_tensor("wu", [EPC, HID, INTER], BF16, kind="ExternalInput")
    wd_d = nc.dram_tensor("wd", [EPC, INTER, HID], BF16, kind="ExternalInput")
    shg_d = nc.dram_tensor("shg", [HID, ISH // NC], BF16, kind="ExternalInput")
    shu_d = nc.dram_tensor("shu", [HID, ISH // NC], BF16, kind="ExternalInput")
    shd_d = nc.dram_tensor("shd", [ISH // NC, HID], BF16, kind="ExternalInput")
    ident_d = nc.dram_tensor("ident", [128, 128], FP32, kind="ExternalInput")
    ones_d = nc.dram_tensor("ones128", [128, 128], BF16, kind="ExternalInput")
    triu_d = nc.dram_tensor("triu128", [128, 128], BF16, kind="ExternalInput")
    esel_d = nc.dram_tensor("esel", [128, EPC, E], FP32,
                            kind="ExternalInput")
    payhl_d = nc.dram_tensor("payhl", [128, NTT], F16,
                             kind="ExternalInput")
    iota_d = nc.dram_tensor("iota_nt", [128, NTT, 128], F16,
                            kind="ExternalInput")
    tokpay_d = nc.dram_tensor("tokpay", [128, PAY_S], FP32,
                              kind="ExternalInput")
    out_d = nc.dram_tensor("out", [T // NC, HID], FP32, kind="ExternalOutput")

    ISH_C = ISH // NC                       # 176
    SH_IC = [(128, 0), (48, 128)]           # shared inter chunks

    with tile.TileContext(nc) as tc:
        with (
            tc.tile_pool(name="pers", bufs=1) as pers,
            tc.tile_pool(name="dram", bufs=1, space="DRAM") as dram,
        ):
            # ---- constants ----
            ident = pers.tile([128, 128], FP32, tag="ident")
            nc.sync.dma_start(ident[:], ident_d[:])
            ones_sb = pers.tile([128, 128], BF16, tag="ones")
            nc.sync.dma_start(ones_sb[:], ones_d[:])
            triu_sb = pers.tile([128, 128], BF16, tag="triu")
            nc.sync.dma_start(triu_sb[:], triu_d[:])
            tokpay_sb = pers.tile([128, PAY_S], FP32, tag="tokpay")
            nc.sync.dma_start(tokpay_sb[:], tokpay_d[:])

            # routing-lifetime constants (freed before the expert phase)
            cstp = tc.alloc_tile_pool(name="cst", bufs=1)
            gw_sb = cstp.tile([128, NH, E], FP32, tag="gw")
            nc.sync.dma_start(gw_sb[:], gwT_d.ap().transpose([1, 0, 2]))
            bias_sb = cstp.tile([128, E], FP32, tag="bias")
            nc.sync.dma_start(bias_sb[:], bias_d[:])
            esel_sb = cstp.tile([128, EPC, E], FP32, tag="esel")
            nc.sync.dma_start(esel_sb[:], esel_d[:])
            payhl_sb = cstp.tile([128, NTT], F16, tag="payhl")
            nc.sync.dma_start(payhl_sb[:], payhl_d[:])
            iota_sb = cstp.tile([128, NTT, 128], F16, tag="iotant")
            nc.sync.dma_start(iota_sb[:], iota_d[:])

            # shared-expert weights (bf16, small; right stack so their
            # lifetime can span the routing section independently). xtb is
            # the bf16 copy of x^T the shared experts consume.
            shwp = tc.alloc_tile_pool(name="shw", bufs=1, side="right")
            shg_sb = shwp.tile([128, NH, ISH_C], BF16, tag="shg")
            nc.sync.dma_start(
                shg_sb[:], shg_d.ap().rearrange("(j p) i -> p j i", p=128))
            shu_sb = shwp.tile([128, NH, ISH_C], BF16, tag="shu")
            nc.sync.dma_start(
                shu_sb[:], shu_d.ap().rearrange("(j p) i -> p j i", p=128))
            shd_sb = shwp.tile([128, 2, HID], BF16, tag="shd")
            nc.sync.dma_start(shd_sb[0:128, 0, :], shd_d[0:128, :])
            nc.sync.dma_start(shd_sb[0:48, 1, :], shd_d[128:176, :])
            xtb = shwp.tile([128, NH, T], BF16, tag="xtb")

            # routing results that outlive the routing scope
            tok_rep = pers.tile([128, s_core // 16], I16, tag="tokr")
            w_slots = pers.tile([128, s_core // 128], FP32, tag="wslots")

            # internal DRAM
            tok_dram = dram.tile([1, s_core], I16)
            w_dram = dram.tile([1, s_core], FP32)
            out_part = dram.tile([T, HID], OUT_DT)
            rs_out = dram.tile([T // NC, HID], OUT_DT)

            # ---- 1. load + transpose x, fp32 gate logits on the fly ----
            scores = cstp.tile([128, NTT, E], FP32, tag="scores")
            xtp = tc.alloc_tile_pool(name="xT", bufs=1, side="right")
            xT = xtp.tile([128, NH, T], FP32R, tag="xTall")
            psLo = tc.alloc_tile_pool(name="psLo", bufs=2, space="PSUM")
            iop = tc.alloc_tile_pool(name="iop", bufs=3)
            lop = tc.alloc_tile_pool(name="lop", bufs=2)
            for i in range(NTT):
                xt = iop.tile([128, HID], FP32, tag="xin")
                nc.sync.dma_start(xt[:], x_d[128 * i:128 * (i + 1), :])
                for a in range(2):
                    ptq = psLo.tile([128, 512], FP32, tag="ptq")
                    for b in range(4):
                        j = 4 * a + b
                        nc.tensor.transpose(
                            ptq[:, 128 * b:128 * (b + 1)],
                            xt[:, 128 * j:128 * (j + 1)], ident[:])
                    src = ptq[:].rearrange("p (b q) -> p b q", b=4)
                    dst = xT[:, 4 * a:4 * a + 4, 128 * i:128 * (i + 1)]
                    dstb = xtb[:, 4 * a:4 * a + 4, 128 * i:128 * (i + 1)]
                    if a == 0:
                        nc.scalar.copy(dst, src)
                        nc.vector.tensor_copy(dstb, src)
                    else:
                        nc.vector.tensor_copy(dst, src)
                        nc.scalar.copy(dstb, src)
                # gate logits per 512-token block, transposed layout:
                # plT[E, 512] = gw.T @ xT (fp32r, full-rate moving dim)
                if i % 4 == 3:
                    tb = i // 4
                    plT = psLo.tile([64, 512], FP32, tag="plT")
                    for j in range(NH):
                        nc.tensor.matmul(
                            plT[:], gw_sb[:, j, :],
                            xT[:, j, 512 * tb:512 * (tb + 1)]
                            .bitcast(FP32),
                            start=(j == 0), stop=(j == NH - 1))
                    pls = lop.tile([64, 512], FP32, tag="plsb")
                    nc.scalar.copy(pls[:], plT[:])
                    ptr = psLo.tile([128, 4, E], FP32, tag="pltr")
                    for q in range(4):
                        nc.tensor.transpose(
                            ptr[:, q, :], pls[:, 128 * q:128 * (q + 1)],
                            ident[0:64, 0:64])
                    nc.scalar.activation(scores[:, 4 * tb:4 * tb + 4, :],
                                         ptr[:], ACTF.Sigmoid)
            lop.release()
            iop.release()
            psLo.release()
            xtp.release()

            # ---- 2. expert weight loads: e0/e1 prefetched now, the rest
            #         emitted from inside the expert loop (distance-2) so the
            #         in-order SP queue can't deadlock on buffer reuse ----
            # PSUM for the routing rank matmul + slot-table matmuls
            psR = tc.alloc_tile_pool(name="psR", bufs=2, space="PSUM")
            psT = tc.alloc_tile_pool(name="psT", bufs=2, space="PSUM")

            # ---- 3. shared experts: emitted in per-tb pieces at seams
            #         inside the routing emission, so shared's DVE multiply
            #         doesn't head-of-line-block routing's DVE stream ----
            shhp = tc.alloc_tile_pool(name="shh", bufs=3)
            stpB = tc.alloc_tile_pool(name="stB", bufs=2)
            psB = tc.alloc_tile_pool(name="psB", bufs=1, space="PSUM")
            psBy = tc.alloc_tile_pool(name="psBy", bufs=2, space="PSUM")

            def emit_shared_tb(tb):
                hs_t = []
                for ci, (csz, coff) in enumerate(SH_IC):
                    pg = psB.tile([128, 512], FP32, tag="shpg")
                    pu = psB.tile([128, 512], FP32, tag="shpu")
                    for j in range(NH):
                        rhs = xtb[:, j, 512 * tb:512 * (tb + 1)]
                        nc.tensor.matmul(
                            pg[0:csz, :],
                            shg_sb[:, j, coff:coff + csz],
                            rhs, start=(j == 0), stop=(j == NH - 1))
                        nc.tensor.matmul(
                            pu[0:csz, :],
                            shu_sb[:, j, coff:coff + csz],
                            rhs, start=(j == 0), stop=(j == NH - 1))
                    hst = shhp.tile([128, 512], BF16, tag="hsh")
                    tmp_s = shhp.tile([128, 512], BF16, tag="hsilu")
                    nc.scalar.activation(tmp_s[0:csz, :], pg[0:csz, :],
                                         ACTF.Silu)
                    nc.vector.tensor_tensor(hst[0:csz, :], tmp_s[0:csz, :],
                                            pu[0:csz, :], ALU.mult)
                    hs_t.append((hst, csz))
                for st in range(4):
                    for nh2 in range(2):
                        py = psBy.tile([128, 512], FP32, tag="shpy")
                        for ci, ((hst, csz), _) in enumerate(
                                zip(hs_t, SH_IC)):
                            nc.tensor.matmul(
                                py[:],
                                hst[0:csz, 128 * st:128 * (st + 1)],
                                shd_sb[0:csz, ci, 512 * nh2:512 * (nh2 + 1)],
                                start=(ci == 0), stop=(ci == 1))
                        ot = stpB.tile([128, 512], OUT_DT, tag="osh")
                        nc.scalar.copy(ot[:], py[:])
                        r0 = 512 * tb + 128 * st
                        nc.sync.dma_start(
                            out_part[r0:r0 + 128,
                                     512 * nh2:512 * (nh2 + 1)], ot[:])

            # ---- 4. routing (DVE + Pool), shared-expert tb blocks emitted
            #         at seams so PE/Act work interleaves with routing DVE --
            rp = tc.alloc_tile_pool(name="rout", bufs=1)
            self_routing(nc, tc, rp, psR, psT, scores, bias_sb,
                         esel_sb, payhl_sb, iota_sb, ones_sb,
                         triu_sb, tok_rep, w_slots, tok_dram,
                         w_dram, cap_sched, s_core,
                         seams=[lambda t=t: emit_shared_tb(t)
                                for t in range(4)])
            rp.release()
            psBy.release()
            psB.release()
            stpB.release()
            shhp.release()
            psT.release()
            psR.release()
            shwp.release()
            cstp.release()

            # ---- 2. expert weight pools (allocated after routing so the
            #         routing-phase pools get the SBUF; e0/e1 loads issue
            #         now and overlap the first gather granules) ----
            wgp = tc.alloc_tile_pool(name="wgp", bufs=2)
            wup = tc.alloc_tile_pool(name="wup", bufs=2)
            wdp = tc.alloc_tile_pool(name="wdp", bufs=2)
            wg_t, wu_t, wd_t = {}, {}, {}

            def emit_weight_load(e):
                wge = wgp.tile([128, NH, INTER], BF16, tag="wge",
                               name=f"wge_{e}")
                nc.sync.dma_start(
                    wge[:], wg_d[e].rearrange("(j p) i -> p j i", p=128))
                wue = wup.tile([128, NH, INTER], BF16, tag="wue",
                               name=f"wue_{e}")
                nc.sync.dma_start(
                    wue[:], wu_d[e].rearrange("(j p) i -> p j i", p=128))
                wde = wdp.tile([128, 6, HID], BF16, tag="wde",
                               name=f"wde_{e}")
                for ci, (csz, coff) in enumerate(zip(IC_SIZES, IC_OFFS)):
                    nc.sync.dma_start(wde[0:csz, ci, :],
                                      wd_d[e, coff:coff + csz, :])
                wg_t[e] = wge
                wu_t[e] = wue
                wd_t[e] = wde

            emit_weight_load(0)
            emit_weight_load(1)

            # ---- 5. gather x^T (bf16) interleaved with expert compute ----
            xhp = tc.alloc_tile_pool(name="xhat", bufs=1)
            xhat = xhp.tile([128, NH, s_core], BF16, tag="xhall")
            psG = tc.alloc_tile_pool(name="psG", bufs=2, space="PSUM")
            iog = tc.alloc_tile_pool(name="iog", bufs=3)

            wp2 = tc.alloc_tile_pool(name="hp", bufs=1)
            stpE = tc.alloc_tile_pool(name="stE", bufs=3)
            psE = tc.alloc_tile_pool(name="psE", bufs=2, space="PSUM")
            psEy = tc.alloc_tile_pool(name="psEy", bufs=2, space="PSUM")

            # 256-row gather granules (tail may be 128)
            gchunks = []
            off = 0
            while off < s_core:
                rows = min(256, s_core - off)
                gchunks.append((off, rows))
                off += rows
            ngch = len(gchunks)

            gsems = []
            grows = []
            issued = [0]
            emitted = [0]          # granules fully transposed so far
            ysems = []
            stg_ring = []

            def issue_gather(c):
                goff, rows = gchunks[c]
                grow = iog.tile([128, 2, HID], FP32, tag="grow")
                gs_ = nc.alloc_semaphore(f"gx{c}")
                gp_ = nc.alloc_semaphore(f"gxp{c}")
                nc.gpsimd.dma_gather(
                    grow[:, 0:rows // 128, :], x_d[:],
                    tok_rep[:, goff // 16:(goff + rows) // 16],
                    rows, rows, HID, elem_step=HID,
                    prepare_only=True, sem=gs_,
                    queue_num=1).then_inc(gp_, 1)
                nc.gpsimd.wait_ge(gp_, 1)
                nc.gpsimd.trigger_dma(1, queue_num=1)
                gsems.append(gs_)
                grows.append(grow)
                issued[0] = c + 1

            def emit_chunk(c):
                # one critical per granule: issue ahead, then wait c's data
                with tc.tile_critical():
                    if c == 0:
                        issue_gather(0)
                    if issued[0] < ngch and issued[0] <= c + 1:
                        issue_gather(issued[0])
                    nc.gpsimd.wait_ge(gsems[c], 16)
                    nc.gpsimd.tensor_copy(grows[c][0:1, 0, 0:1],
                                          grows[c][0:1, 0, 0:1])
                goff, rows = gchunks[c]
                for s in range(rows // 128):
                    for a in range(2):
                        ptg = psG.tile([128, 512], FP32, tag="ptg")
                        for b in range(4):
                            j = 4 * a + b
                            nc.tensor.transpose(
                                ptg[:, 128 * b:128 * (b + 1)],
                                grows[c][:, s, 128 * j:128 * (j + 1)],
                                ident[:])
                        dst = xhat[:, 4 * a:4 * a + 4,
                                   goff + 128 * s:goff + 128 * (s + 1)]
                        src = ptg[:].rearrange("p (b q) -> p b q", b=4)
                        nc.vector.tensor_copy(dst, src)

            def ensure_chunks(nslots):
                # make sure slots [0, nslots) are gathered + transposed
                while emitted[0] < ngch and \
                        gchunks[emitted[0]][0] < nslots:
                    emit_chunk(emitted[0])
                    emitted[0] += 1

            CAPMAX = max(cap_sched)

            def emit_expert(e, lbase):
                cap = cap_sched[e]
                sizes = _blocks(cap)
                if e + 2 < EPC:
                    emit_weight_load(e + 2)
                wge, wue, wde = wg_t[e], wu_t[e], wd_t[e]
                He = [wp2.tile([128, CAPMAX], BF16, tag=f"he{ci}", bufs=2,
                               name=f"he{ci}_{e}") for ci in range(6)]
                for ci, (csz, coff) in enumerate(zip(IC_SIZES, IC_OFFS)):
                    boff = 0
                    for bsz in sizes:
                        bl = lbase + boff
                        pg = psE.tile([128, 512], FP32, tag="epg")
                        pu = psE.tile([128, 512], FP32, tag="epu")
                        for kk in range(NH):
                            nc.tensor.matmul(
                                pg[0:csz, 0:bsz],
                                wge[:, kk, coff:coff + csz],
                                xhat[:, kk, bl:bl + bsz],
                                start=(kk == 0), stop=(kk == NH - 1))
                        for kk in range(NH):
                            nc.tensor.matmul(
                                pu[0:csz, 0:bsz],
                                wue[:, kk, coff:coff + csz],
                                xhat[:, kk, bl:bl + bsz],
                                start=(kk == 0), stop=(kk == NH - 1))
                        hs = wp2.tile([128, 512], BF16, tag="hsil", bufs=3)
                        nc.scalar.activation(
                            hs[0:csz, 0:bsz], pg[0:csz, 0:bsz], ACTF.Silu)
                        nc.vector.tensor_tensor(
                            He[ci][0:csz, boff:boff + bsz],
                            hs[0:csz, 0:bsz],
                            pu[0:csz, 0:bsz], ALU.mult)
                        boff += bsz

                boff = 0
                for bsz in sizes:
                    bl = lbase + boff
                    if len(ysems) >= 3:
                        # stg ring anti-race: re-use of a stg buffer must
                        # wait for the scatter 3 launches back
                        old_sem, old_stg = stg_ring[len(ysems) - 3]
                        with tc.tile_critical():
                            nc.gpsimd.wait_ge(old_sem, 16)
                            nc.gpsimd.tensor_copy(old_stg[0:1, 0, 0:1],
                                                  old_stg[0:1, 0, 0:1])
                    stg = stpE.tile([128, 4, HID], OUT_DT, tag="ystg")
                    for sc_i in range(bsz // 128):
                        so = boff + 128 * sc_i
                        col = (bl + 128 * sc_i) // 128
                        for nh2 in range(2):
                            py = psEy.tile([128, 512], FP32, tag="epy")
                            for ci, csz in enumerate(IC_SIZES):
                                nc.tensor.matmul(
                                    py[:],
                                    He[ci][0:csz, so:so + 128],
                                    wde[0:csz, ci,
                                        512 * nh2:512 * (nh2 + 1)],
                                    start=(ci == 0), stop=(ci == 5))
                            nc.scalar.mul(
                                stg[:, sc_i, 512 * nh2:512 * (nh2 + 1)],
                                py[:], w_slots[:, col:col + 1])
                    ysem = nc.alloc_semaphore(f"swdge_y{e}_{boff}")
                    nc.gpsimd.dma_scatter_add(
                        out_part[:], stg[:, 0:bsz // 128, :],
                        tok_rep[:, bl // 16:(bl + bsz) // 16],
                        bsz, bsz, HID,
                        prepare_only=True, sem=ysem)
                    nc.gpsimd.trigger_dma(count=None)
                    ysems.append(ysem)
                    stg_ring.append((ysem, stg))
                    boff += bsz

            lbase = 0
            for e in range(EPC):
                cap = cap_sched[e]
                ensure_chunks(lbase + cap)
                emit_expert(e, lbase)
                lbase += cap

            psEy.release()
            psE.release()
            stpE.release()
            wp2.release()
            iog.release()
            psG.release()
            xhp.release()
            wdp.release()
            wup.release()
            wgp.release()

            # ---- 6. drain scatters, chunked reduce-scatter, output ----
            with tc.tile_critical():
                for ys in ysems:
                    nc.gpsimd.wait_ge(ys, 16)
                d2sem = nc.alloc_semaphore("y_drain")
                nc.gpsimd.dma_start(
                    tokpay_sb[0:1, 0:1].bitcast(OUT_DT)[0:1, 0:1],
                    out_part[0:1, 0:1]).then_inc(d2sem, 16)
                nc.gpsimd.wait_ge(d2sem, 16)
            iop2 = tc.alloc_tile_pool(name="iop2", bufs=2)
            if COMB == "a2a":
                # mesh AllToAll (concurrent peer streams, unlike the ~20GB/s
                # RDH ring RS), then an on-chip 8-shard reduce. Core c
                # receives shard c of every rank's out_part: chunk j holds
                # rank j's partial rows for tokens [256c, 256c+256).
                a2a_out = dram.tile([T, HID], OUT_DT)
                nc.gpsimd.collective_compute(
                    "AllToAll", ALU.bypass,
                    replica_groups=[list(range(NC))],
                    ins=[out_part.opt()], outs=[a2a_out.opt()])
                shp = tc.alloc_tile_pool(name="shrd", bufs=4)
                TPC = T // NC           # 256 tokens per core
                for i in range(TPC // 128):
                    acc = iop2.tile([128, HID], FP32, tag="acc")
                    for j in range(NC):
                        sh_t = shp.tile([128, HID], OUT_DT, tag="sha")
                        nc.sync.dma_start(
                            sh_t[:],
                            a2a_out[TPC * j + 128 * i:
                                    TPC * j + 128 * (i + 1), :])
                        if j == 0:
                            nc.vector.tensor_copy(acc[:], sh_t[:])
                        else:
                            nc.vector.tensor_tensor(acc[:], acc[:],
                                                    sh_t[:], ALU.add)
                    nc.sync.dma_start(out_d[128 * i:128 * (i + 1), :],
                                      acc[:])
                shp.release()
            else:
                # RS in RS_CH chunks of [T/RS_CH, HID]; core c's chunk
                # shard is tokens [T/RS_CH*ch + T/NC/RS_CH*c, ...).
                # kernel() reassembles rows via unshard_out.
                TCH = T // RS_CH            # tokens per chunk
                OCH = TCH // NC             # out rows per chunk per core
                for ch in range(RS_CH):
                    nc.gpsimd.collective_compute(
                        "ReduceScatter", ALU.add,
                        replica_groups=[list(range(NC))],
                        ins=[out_part[TCH * ch:TCH * (ch + 1), :].opt()],
                        outs=[rs_out[OCH * ch:OCH * (ch + 1), :].opt()])
                for i in range(T // NC // 128):
                    ot2 = iop2.tile([128, HID], OUT_DT, tag="outld")
                    nc.sync.dma_start(ot2[:],
                                      rs_out[128 * i:128 * (i + 1), :])
                    of2 = iop2.tile([128, HID], FP32, tag="outf")
                    nc.scalar.copy(of2[:], ot2[:])
                    nc.sync.dma_start(out_d[128 * i:128 * (i + 1), :],
                                      of2[:])
            iop2.release()

    nc.compile()
    return nc


def self_routing(nc, tc, rp, psR, psT, scores, bias_sb, esel_sb, payhl_sb,
                 iota_sb, ones_sb, triu_sb, tok_rep, w_slots, tok_dram,
                 w_dram, cap_sched, s_core, seams=()):
    """Grouped top-k -> per-expert rank -> ON-CHIP slot table.

    Instead of scattering (t,k) payloads to HBM and reading back, the
    compact per-slot (token, weight) table is built on-chip: per expert
    position, a one-hot over rank (is_eq vs iota) is contracted with a
    (tok_hi, tok_lo, w) payload on the PE; tok = 128*hi + lo keeps token
    ids exact in bf16. `seams` are emission callbacks (shared-expert tb
    blocks) interleaved at points where routing's DVE chain has gaps."""
    V = nc.vector
    P = nc.gpsimd
    seams = list(seams)

    def seam():
        if seams:
            seams.pop(0)()

    # transient routing tiles, freed before the table-build pools
    rt = tc.alloc_tile_pool(name="rtmp", bufs=1)

    seam()
    sc_b = rt.tile([128, NTT, E], FP32, tag="scb")
    V.tensor_tensor(
        sc_b[:], scores[:],
        bias_sb[:].unsqueeze(1).to_broadcast([128, NTT, E]), ALU.add)

    scg = sc_b[:].rearrange("p t (g s) -> p t g s", g=NG, s=GS)
    m1 = rt.tile([128, NTT, NG], FP32, tag="m1")
    V.tensor_reduce(m1[:], scg, AXL.X, ALU.max)
    oh1 = rt.tile([128, NTT, NG, GS], FP32, tag="oh1")
    V.tensor_tensor(
        oh1[:], scg,
        m1[:].unsqueeze(3).to_broadcast([128, NTT, NG, GS]), ALU.is_ge)
    # masked second-max, in place over oh1
    V.scalar_tensor_tensor(oh1[:], oh1[:], NEG, scg, ALU.mult, ALU.add)
    m2 = rt.tile([128, NTT, NG], FP32, tag="m2")
    V.tensor_reduce(m2[:], oh1[:], AXL.X, ALU.max)
    gsc = rt.tile([128, NTT, NG], FP32, tag="gsc")
    V.tensor_tensor(gsc[:], m1[:], m2[:], ALU.add)

    seam()
    gmask = rt.tile([128, NTT, NG], FP32, tag="gmask")
    P.memset(gmask[:], 0.0)
    for g in range(TG):
        gm = rt.tile([128, NTT, 1], FP32, tag="gm")
        V.tensor_reduce(gm[:], gsc[:], AXL.X, ALU.max)
        ohg = rt.tile([128, NTT, NG], FP32, tag="ohg")
        V.tensor_tensor(ohg[:], gsc[:],
                        gm[:].to_broadcast([128, NTT, NG]), ALU.is_ge)
        V.tensor_tensor(gmask[:], gmask[:], ohg[:], ALU.add)
        V.scalar_tensor_tensor(gsc[:], ohg[:], NEG, gsc[:],
                               ALU.mult, ALU.add)

    # masked = sc where group selected else -1e30, built in place over sc_b
    masked = sc_b
    mview = scg
    V.tensor_tensor(
        mview, scg,
        gmask[:].unsqueeze(3).to_broadcast([128, NTT, NG, GS]), ALU.mult)
    gb = rt.tile([128, NTT, NG], FP32, tag="gb")
    V.tensor_scalar(gb[:], gmask[:], 1.0e30, -1.0e30, ALU.mult, ALU.add)
    V.tensor_tensor(
        mview, mview,
        gb[:].unsqueeze(3).to_broadcast([128, NTT, NG, GS]), ALU.add)

    seam()
    ohall = rt.tile([128, K, NTT, E], BF16, tag="ohall")
    for k in range(K):
        mk = rt.tile([128, NTT, 1], FP32, tag=f"mk{k}")
        V.tensor_reduce(mk[:], masked[:], AXL.X, ALU.max)
        ohk = ohall[:, k, :, :]
        V.tensor_tensor(ohk, masked[:],
                        mk[:].to_broadcast([128, NTT, E]), ALU.is_ge)
        V.scalar_tensor_tensor(masked[:], ohk, NEG, masked[:],
                               ALU.mult, ALU.add)

    seam()
    # rank matmul (bf16 0/1 inputs, fp32 psum), 4 token-tiles per PSUM bank
    msel = rt.tile([128, NTT, E], FP32, tag="msel")
    V.tensor_reduce(msel[:],
                    ohall[:].rearrange("p k t e -> p t e k"),
                    AXL.X, ALU.add)
    msel_bf = rt.tile([128, NTT, E], BF16, tag="mselbf")
    V.tensor_copy(msel_bf[:], msel[:])
    R = rt.tile([128, NTT, E], FP32, tag="R")
    for q in range(NTT // 4):
        pr = psR.tile([128, 4, E], FP32, tag="prq")
        for ii in range(4):
            i = 4 * q + ii
            n_mm = i + 1
            for mi in range(n_mm):
                lhsT = ones_sb[:] if mi < i else triu_sb[:]
                nc.tensor.matmul(pr[:, ii, :], lhsT, msel_bf[:, mi, :],
                                 start=(mi == 0), stop=(mi == n_mm - 1))
        V.tensor_copy(R[:, 4 * q:4 * q + 4, :], pr[:])

    # ---- W[t,e]: normalized expert weight x SCALE (0 if not selected) ----
    BIG = 1.0e9
    RmW = rp.tile([128, 2, NTT, E], FP32, tag="RmW")
    SW = rt.tile([128, NTT, E], FP32, tag="SW")
    V.tensor_tensor(SW[:], msel[:], scores[:], ALU.mult)
    den = rt.tile([128, NTT], FP32, tag="den")
    V.tensor_reduce(den[:], SW[:], AXL.X, ALU.add)
    rden = rt.tile([128, NTT], FP32, tag="rden")
    V.reciprocal(rden[:], den[:])
    rdenS = rt.tile([128, NTT], FP32, tag="rdenS")
    V.tensor_scalar_mul(rdenS[:], rden[:], SCALE)
    V.tensor_tensor(RmW[:, 1, :, :], SW[:],
                    rdenS[:].unsqueeze(2).to_broadcast([128, NTT, E]),
                    ALU.mult)

    # ---- Rm[t,e]: rank where selected, else BIG ----
    msel_u8 = rt.tile([128, NTT, E], mybir.dt.uint8, tag="mselu")
    V.tensor_copy(msel_u8[:], msel[:])
    P.memset(RmW[:, 0, :, :], BIG)
    V.copy_predicated(RmW[:, 0, :, :], msel_u8[:], R[:])
    rt.release()

    # ---- per expert position: one-hot over rank -> table matmul ----
    # fp16 keeps token ids (<= 2047) and 0/1 one-hots exact at full PE rate
    tab = rp.tile([2, s_core], FP32, tag="tab")
    tok16 = rp.tile([1, s_core], I16, tag="tok16")

    def emit_table_out(c0, c1):
        # convert + DRAM roundtrip into the wrapped idx layout for slot
        # columns [c0, c1); emitted per chunk so the first gathers can
        # start while later positions are still building their table
        V.tensor_copy(tok16[0:1, c0:c1], tab[0:1, c0:c1])
        nc.sync.dma_start(tok_dram[0:1, c0:c1], tok16[0:1, c0:c1])
        nc.sync.dma_start(w_dram[0:1, c0:c1], tab[1:2, c0:c1])
        nc.sync.dma_start(
            w_slots[:, c0 // 128:c1 // 128],
            w_dram[0, c0:c1].rearrange("(c p) -> p c", p=128))
        nc.sync.dma_start(
            tok_rep[0:16, c0 // 16:c1 // 16],
            tok_dram[0, c0:c1].rearrange("(s q) -> q s", q=16))
        for b in range(1, 8):
            nc.sync.dma_start(
                tok_rep[16 * b:16 * (b + 1), c0 // 16:c1 // 16],
                tok_rep[0:16, c0 // 16:c1 // 16])

    selp = tc.alloc_tile_pool(name="selp", bufs=2)
    ohp = tc.alloc_tile_pool(name="ohp", bufs=2)
    split = cap_sched[0] + cap_sched[1]
    lbase = 0
    for p in range(EPC):
        cap = cap_sched[p]
        tmp_pe = selp.tile([128, 2, NTT, E], FP32, tag="tmpPE")
        P.tensor_tensor(
            tmp_pe[:], RmW[:],
            esel_sb[:, p, :].unsqueeze(1).unsqueeze(1)
            .to_broadcast([128, 2, NTT, E]), ALU.mult)
        rmw_p = selp.tile([128, 2, NTT], FP32, tag="rmwp")
        V.tensor_reduce(rmw_p[:], tmp_pe[:], AXL.X, ALU.add)
        pay2 = selp.tile([128, NTT, 2], F16, tag="pay2")
        V.tensor_copy(pay2[:, :, 0], payhl_sb[:])
        V.tensor_copy(pay2[:, :, 1], rmw_p[:, 1, :])
        for B in range(cap // 128):
            rmb = selp.tile([128, NTT], F16, tag="rmb")
            V.tensor_scalar_add(rmb[:], rmw_p[:, 0, :], float(-128 * B))
            ohb = ohp.tile([128, NTT, 128], F16, tag="ohb")
            V.tensor_tensor(
                ohb[:], iota_sb[:],
                rmb[:].unsqueeze(2).to_broadcast([128, NTT, 128]),
                ALU.is_equal)
            pt = psT.tile([2, 128], FP32, tag="ptab")
            for i in range(NTT):
                nc.tensor.matmul(pt[:], pay2[:, i, :],
                                 ohb[:, i, :],
                                 start=(i == 0), stop=(i == NTT - 1))
            soff = lbase + 128 * B
            nc.scalar.copy(tab[:, soff:soff + 128], pt[:])
        lbase += cap
        if p == 1:
            emit_table_out(0, split)
    ohp.release()
    selp.release()

    # unfilled slots give tok=0, w=0 (harmless)
    emit_table_out(split, s_core)


def prepare(**inputs):
    import ml_dtypes

    x = np.ascontiguousarray(inputs["hidden_states"], dtype=np.float32)
    gate_w = np.ascontiguousarray(inputs["gate_w"], dtype=np.float32)
    score_bias = np.ascontiguousarray(inputs["score_bias"], dtype=np.float32)

    cnt = _host_routing_counts(x, gate_w, score_bias)
    caps = ((cnt + 16 + 127) // 128 * 128).astype(np.int64)
    bins, cap_sched = _placement(caps)
    s_core = int(sum(cap_sched))

    ident = np.eye(128, dtype=np.float32)
    ones128 = np.ones((128, 128), dtype=ml_dtypes.bfloat16)
    triu128 = np.triu(np.ones((128, 128), np.float32), 1).astype(
        ml_dtypes.bfloat16)
    tokpay = np.zeros((128, PAY_S), np.float32)
    for k in range(K):
        for i in range(NTT):
            tokpay[:, k * NTT + i] = np.arange(128) + 128 * i

    bf = ml_dtypes.bfloat16
    # token-id payload (fp16 holds ints <= 2048 exactly)
    payhl = np.zeros((128, NTT), np.float16)
    for i in range(NTT):
        payhl[:, i] = np.arange(128) + 128 * i
    iota_nt = np.tile(np.arange(128, dtype=np.float16)[None, None, :],
                      (128, NTT, 1))

    ish_c = ISH // NC
    in_maps = []
    for c in range(NC):
        esel = np.zeros((128, EPC, E), np.float32)
        for p, e in enumerate(bins[c]):
            esel[:, p, e] = 1.0
        perm = bins[c]
        in_maps.append({
            "x": x,
            "gwT": np.ascontiguousarray(gate_w.T.reshape(NH, 128, E)),
            "bias_b": np.ascontiguousarray(
                np.tile(score_bias, (128, 1))),
            "wg": np.ascontiguousarray(
                np.asarray(inputs["w_gate"], np.float32)[perm].astype(bf)),
            "wu": np.ascontiguousarray(
                np.asarray(inputs["w_up"], np.float32)[perm].astype(bf)),
            "wd": np.ascontiguousarray(
                np.asarray(inputs["w_down"], np.float32)[perm].astype(bf)),
            "shg": np.ascontiguousarray(
                np.asarray(inputs["sh_gate"],
                           np.float32)[:, c * ish_c:(c + 1) * ish_c]
                .astype(bf)),
            "shu": np.ascontiguousarray(
                np.asarray(inputs["sh_up"],
                           np.float32)[:, c * ish_c:(c + 1) * ish_c]
                .astype(bf)),
            "shd": np.ascontiguousarray(
                np.asarray(inputs["sh_down"],
                           np.float32)[c * ish_c:(c + 1) * ish_c, :]
                .astype(bf)),
            "ident": ident,
            "ones128": ones128,
            "triu128": triu128,
            "esel": esel,
            "payhl": np.ascontiguousarray(payhl),
            "iota_nt": np.ascontiguousarray(iota_nt),
            "tokpay": tokpay,
        })

    nc = build_graph(cap_sched, s_core)
    return nc, in_maps


def unshard_out(per_core):
    """Chunked-RS row mapping: core c's out rows [OCH*ch, OCH*(ch+1)) are
    global tokens [TCH*ch + OCH*c, ...)."""
    TCH = T // RS_CH
    OCH = TCH // NC
    out = np.empty((T, HID), np.float32)
    for c in range(NC):
        r = per_core[c]
        for ch in range(RS_CH):
            out[TCH * ch + OCH * c:TCH * ch + OCH * (c + 1)] = \
                r[OCH * ch:OCH * (ch + 1)]
    return out


def kernel(**inputs):
    nc, in_maps = prepare(**inputs)
    res = run_bass_kernel_spmd(nc, in_maps, core_ids=list(range(NC)))
    out = unshard_out([res.results[c]["out"] for c in range(NC)])
    return np.ascontiguousarray(out, dtype=np.float32)


if __name__ == "__main__":
    import reference
    inp = {k: np.asarray(v) for k, v in reference.setup_inputs().items()}
    out = kernel(**inp)
    print("out", out.shape, out.dtype)



# revision 55
# speedup vs baseline: 1.7993x; 1.7993x over previous
"""DeepseekV2-style MoE (64 experts, top-6 grouped sigmoid routing) on 8 TRN2
NeuronCores — expert-parallel, on-chip routing table.

Structure (HW exec ~0.9 ms/core, amortized ~1.25 ms/call):
- bf16 weights everywhere (expert + shared MLPs), bf16 dispatched
  activations (xhat/xtb); Silu fused on the Act engine. Routing numerics
  fp32; index/one-hot table matmuls in fp16 (ints <= 2048 exact; bf16
  corrupts ids > 256).
- Shared experts emitted at seams inside the routing emission so their
  PE/Act work overlaps routing's DVE chain.
- Per-expert rank via triangular ones/triu matmuls (PE), then the compact
  per-slot (token, weight) table is built ON-CHIP: is_equal one-hot over
  rank x iota, contracted with a (token_id, weight) fp16 payload on the
  PE; one small DRAM roundtrip rearranges it into the wrapped idx layout
  dma_gather/dma_scatter_add require. (The previous HBM payload
  scatter+readback cost ~200 us serial and ~20k static DMA descriptors,
  which also inflated per-call dispatch by ~0.6 ms.)
- 256-row gather granules pipelined with expert compute; 512-wide expert
  MLP blocks; weights streamed distance-2 on the SP HWDGE ring (moving
  wd loads to the ACT ring measured slower).
- out_part + single ReduceScatter in bf16 (chunked RS and mesh
  AllToAll+local-reduce both measured slower), final convert to fp32.
"""

import sys

sys.path.insert(0, "/opt/trn_rl_repo")

import numpy as np

from concourse import bacc, tile
import concourse.mybir as mybir
from concourse.bass_utils import run_bass_kernel_spmd

FP32 = mybir.dt.float32
FP32R = mybir.dt.float32r
BF16 = mybir.dt.bfloat16
F16 = mybir.dt.float16
I16 = mybir.dt.int16
ALU = mybir.AluOpType
AXL = mybir.AxisListType
ACTF = mybir.ActivationFunctionType

T = 2048
HID = 1024
E = 64
INTER = 704
K = 6
NG = 8
TG = 4
ISH = 1408
SCALE = 2.5
NC = 8
EPC = 8
NEG = -1.0e30

NTT = T // 128            # 16 token tiles
NH = HID // 128           # 8 hid chunks
IC_SIZES = [128, 128, 128, 128, 128, 64]
IC_OFFS = [0, 128, 256, 384, 512, 640]
GS = E // NG              # 8 experts / group
PAY_S = K * NTT           # 96
PAYW = 64                 # slot-scatter payload width (256B row stride min)

OUT_DT = BF16             # out_part + ReduceScatter dtype (BF16 or FP32)
RS_CH = 1                 # ReduceScatter chunks (chunking measured slower)
COMB = "rs"              # "rs": ncfw ReduceScatter; "a2a": mesh AllToAll
                          # + on-chip 8-shard reduce


def _host_routing_counts(x, gate_w, score_bias):
    logits = x.astype(np.float64) @ gate_w.T.astype(np.float64)
    scores = 1.0 / (1.0 + np.exp(-logits))
    sc = scores + score_bias[None, :]
    gs = sc.reshape(T, NG, GS)
    top2 = np.sort(gs, axis=-1)[:, :, -2:].sum(-1)
    gidx = np.argsort(-top2, axis=-1)[:, :TG]
    gmask = np.zeros((T, NG), np.float64)
    np.put_along_axis(gmask, gidx, 1.0, axis=1)
    smask = np.repeat(gmask, GS, axis=1)
    masked = np.where(smask > 0, sc, -np.inf)
    ids = np.argsort(-masked, axis=-1)[:, :K]
    cnt = np.zeros(E, np.int64)
    for k in range(K):
        cnt += np.bincount(ids[:, k], minlength=E)
    return cnt


def _placement(caps):
    """Rank-based packing: position p holds experts ranked [NC*p, NC*(p+1))."""
    order = np.argsort(-caps)
    bins = [[int(order[NC * p + c]) for p in range(EPC)] for c in range(NC)]
    cap_sched = [int(caps[order[NC * p]]) for p in range(EPC)]
    return bins, cap_sched


def _blocks(cap):
    sizes = [512] * (cap // 512)
    if cap % 512:
        sizes.append(cap % 512)
    return sizes


def build_graph(cap_sched, s_core):
    nc = bacc.Bacc("TRN2", target_bir_lowering=False, debug=False,
                   num_devices=NC, num_swdge_queues=2)

    x_d = nc.dram_tensor("x", [T, HID], FP32, kind="ExternalInput")
    gwT_d = nc.dram_tensor("gwT", [NH, 128, E], FP32, kind="ExternalInput")
    bias_d = nc.dram_tensor("bias_b", [128, E], FP32, kind="ExternalInput")
    wg_d = nc.dram_tensor("wg", [EPC, HID, INTER], BF16, kind="ExternalInput")
    wu_d = nc.dram_tensor("wu", [EPC, HID, INTER], BF16, kind="ExternalInput")
    wd_d = nc.dram_tensor("wd", [EPC, INTER, HID], BF16, kind="ExternalInput")
    shg_d = nc.dram_tensor("shg", [HID, ISH // NC], BF16, kind="ExternalInput")
    shu_d = nc.dram_tensor("shu", [HID, ISH // NC], BF16, kind="ExternalInput")
    shd_d = nc.dram_tensor("shd", [ISH // NC, HID], BF16, kind="ExternalInput")
    ident_d = nc.dram_tensor("ident", [128, 128], FP32, kind="ExternalInput")
    ones_d = nc.dram_tensor("ones128", [128, 128], BF16, kind="ExternalInput")
    triu_d = nc.dram_tensor("triu128", [128, 128], BF16, kind="ExternalInput")
    esel_d = nc.dram_tensor("esel", [128, EPC, E], FP32,
                            kind="ExternalInput")
    payhl_d = nc.dram_tensor("payhl", [128, NTT], F16,
                             kind="ExternalInput")
    iota_d = nc.dram_tensor("iota_nt", [128, NTT, 128], F16,
                            kind="ExternalInput")
    tokpay_d = nc.dram_tensor("tokpay", [128, PAY_S], FP32,
                              kind="ExternalInput")
    out_d = nc.dram_tensor("out", [T // NC, HID], FP32, kind="ExternalOutput")

    ISH_C = ISH // NC                       # 176
    SH_IC = [(128, 0), (48, 128)]           # shared inter chunks

    with tile.TileContext(nc) as tc:
        with (
            tc.tile_pool(name="pers", bufs=1) as pers,
            tc.tile_pool(name="dram", bufs=1, space="DRAM") as dram,
        ):
            # ---- constants ----
            ident = pers.tile([128, 128], FP32, tag="ident")
            nc.sync.dma_start(ident[:], ident_d[:])
            ones_sb = pers.tile([128, 128], BF16, tag="ones")
            nc.sync.dma_start(ones_sb[:], ones_d[:])
            triu_sb = pers.tile([128, 128], BF16, tag="triu")
            nc.sync.dma_start(triu_sb[:], triu_d[:])
            gw_sb = pers.tile([128, NH, E], FP32, tag="gw")
            nc.sync.dma_start(gw_sb[:], gwT_d.ap().transpose([1, 0, 2]))
            bias_sb = pers.tile([128, E], FP32, tag="bias")
            nc.sync.dma_start(bias_sb[:], bias_d[:])
            tokpay_sb = pers.tile([128, PAY_S], FP32, tag="tokpay")
            nc.sync.dma_start(tokpay_sb[:], tokpay_d[:])

            # routing-lifetime constants (freed before the expert phase)
            cstp = tc.alloc_tile_pool(name="cst", bufs=1)
            esel_sb = cstp.tile([128, EPC, E], FP32, tag="esel")
            nc.sync.dma_start(esel_sb[:], esel_d[:])
            payhl_sb = cstp.tile([128, NTT], F16, tag="payhl")
            nc.sync.dma_start(payhl_sb[:], payhl_d[:])
            iota_sb = cstp.tile([128, NTT, 128], F16, tag="iotant")
            nc.sync.dma_start(iota_sb[:], iota_d[:])

            # shared-expert weights (bf16, small; right stack so their
            # lifetime can span the routing section independently). xtb is
            # the bf16 copy of x^T the shared experts consume.
            shwp = tc.alloc_tile_pool(name="shw", bufs=1, side="right")
            shg_sb = shwp.tile([128, NH, ISH_C], BF16, tag="shg")
            nc.sync.dma_start(
                shg_sb[:], shg_d.ap().rearrange("(j p) i -> p j i", p=128))
            shu_sb = shwp.tile([128, NH, ISH_C], BF16, tag="shu")
            nc.sync.dma_start(
                shu_sb[:], shu_d.ap().rearrange("(j p) i -> p j i", p=128))
            shd_sb = shwp.tile([128, 2, HID], BF16, tag="shd")
            nc.sync.dma_start(shd_sb[0:128, 0, :], shd_d[0:128, :])
            nc.sync.dma_start(shd_sb[0:48, 1, :], shd_d[128:176, :])
            xtb = shwp.tile([128, NH, T], BF16, tag="xtb")

            # routing results that outlive the routing scope
            tok_rep = pers.tile([128, s_core // 16], I16, tag="tokr")
            w_slots = pers.tile([128, s_core // 128], FP32, tag="wslots")

            # internal DRAM
            tok_dram = dram.tile([1, s_core], I16)
            w_dram = dram.tile([1, s_core], FP32)
            out_part = dram.tile([T, HID], OUT_DT)
            rs_out = dram.tile([T // NC, HID], OUT_DT)

            # ---- 1. load + transpose x, fp32 gate logits on the fly ----
            scores = cstp.tile([128, NTT, E], FP32, tag="scores")
            xtp = tc.alloc_tile_pool(name="xT", bufs=1, side="right")
            xT = xtp.tile([128, NH, T], FP32R, tag="xTall")
            psLo = tc.alloc_tile_pool(name="psLo", bufs=2, space="PSUM")
            iop = tc.alloc_tile_pool(name="iop", bufs=3)
            lop = tc.alloc_tile_pool(name="lop", bufs=2)
            for i in range(NTT):
                xt = iop.tile([128, HID], FP32, tag="xin")
                nc.sync.dma_start(xt[:], x_d[128 * i:128 * (i + 1), :])
                for a in range(2):
                    ptq = psLo.tile([128, 512], FP32, tag="ptq")
                    for b in range(4):
                        j = 4 * a + b
                        nc.tensor.transpose(
                            ptq[:, 128 * b:128 * (b + 1)],
                            xt[:, 128 * j:128 * (j + 1)], ident[:])
                    src = ptq[:].rearrange("p (b q) -> p b q", b=4)
                    dst = xT[:, 4 * a:4 * a + 4, 128 * i:128 * (i + 1)]
                    dstb = xtb[:, 4 * a:4 * a + 4, 128 * i:128 * (i + 1)]
                    if a == 0:
                        nc.scalar.copy(dst, src)
                        nc.vector.tensor_copy(dstb, src)
                    else:
                        nc.vector.tensor_copy(dst, src)
                        nc.scalar.copy(dstb, src)
                # gate logits per 512-token block, transposed layout:
                # plT[E, 512] = gw.T @ xT (fp32r, full-rate moving dim)
                if i % 4 == 3:
                    tb = i // 4
                    plT = psLo.tile([64, 512], FP32, tag="plT")
                    for j in range(NH):
                        nc.tensor.matmul(
                            plT[:], gw_sb[:, j, :],
                            xT[:, j, 512 * tb:512 * (tb + 1)]
                            .bitcast(FP32),
                            start=(j == 0), stop=(j == NH - 1))
                    pls = lop.tile([64, 512], FP32, tag="plsb")
                    nc.scalar.copy(pls[:], plT[:])
                    ptr = psLo.tile([128, 4, E], FP32, tag="pltr")
                    for q in range(4):
                        nc.tensor.transpose(
                            ptr[:, q, :], pls[:, 128 * q:128 * (q + 1)],
                            ident[0:64, 0:64])
                    nc.scalar.activation(scores[:, 4 * tb:4 * tb + 4, :],
                                         ptr[:], ACTF.Sigmoid)
            lop.release()
            iop.release()
            psLo.release()
            xtp.release()

            # ---- 2. expert weight loads: e0/e1 prefetched now, the rest
            #         emitted from inside the expert loop (distance-2) so the
            #         in-order SP queue can't deadlock on buffer reuse ----
            # PSUM for the routing rank matmul + slot-table matmuls
            psR = tc.alloc_tile_pool(name="psR", bufs=2, space="PSUM")
            psT = tc.alloc_tile_pool(name="psT", bufs=2, space="PSUM")

            # ---- 3. shared experts: emitted in per-tb pieces at seams
            #         inside the routing emission, so shared's DVE multiply
            #         doesn't head-of-line-block routing's DVE stream ----
            shhp = tc.alloc_tile_pool(name="shh", bufs=3)
            stpB = tc.alloc_tile_pool(name="stB", bufs=2)
            psB = tc.alloc_tile_pool(name="psB", bufs=1, space="PSUM")
            psBy = tc.alloc_tile_pool(name="psBy", bufs=2, space="PSUM")

            def emit_shared_tb(tb):
                hs_t = []
                for ci, (csz, coff) in enumerate(SH_IC):
                    pg = psB.tile([128, 512], FP32, tag="shpg")
                    pu = psB.tile([128, 512], FP32, tag="shpu")
                    for j in range(NH):
                        rhs = xtb[:, j, 512 * tb:512 * (tb + 1)]
                        nc.tensor.matmul(
                            pg[0:csz, :],
                            shg_sb[:, j, coff:coff + csz],
                            rhs, start=(j == 0), stop=(j == NH - 1))
                        nc.tensor.matmul(
                            pu[0:csz, :],
                            shu_sb[:, j, coff:coff + csz],
                            rhs, start=(j == 0), stop=(j == NH - 1))
                    hst = shhp.tile([128, 512], BF16, tag="hsh")
                    tmp_s = shhp.tile([128, 512], BF16, tag="hsilu")
                    nc.scalar.activation(tmp_s[0:csz, :], pg[0:csz, :],
                                         ACTF.Silu)
                    nc.vector.tensor_tensor(hst[0:csz, :], tmp_s[0:csz, :],
                                            pu[0:csz, :], ALU.mult)
                    hs_t.append((hst, csz))
                for st in range(4):
                    for nh2 in range(2):
                        py = psBy.tile([128, 512], FP32, tag="shpy")
                        for ci, ((hst, csz), _) in enumerate(
                                zip(hs_t, SH_IC)):
                            nc.tensor.matmul(
                                py[:],
                                hst[0:csz, 128 * st:128 * (st + 1)],
                                shd_sb[0:csz, ci, 512 * nh2:512 * (nh2 + 1)],
                                start=(ci == 0), stop=(ci == 1))
                        ot = stpB.tile([128, 512], OUT_DT, tag="osh")
                        nc.scalar.copy(ot[:], py[:])
                        r0 = 512 * tb + 128 * st
                        nc.sync.dma_start(
                            out_part[r0:r0 + 128,
                                     512 * nh2:512 * (nh2 + 1)], ot[:])

            # ---- 4. routing (DVE + Pool), shared-expert tb blocks emitted
            #         at seams so PE/Act work interleaves with routing DVE --
            rp = tc.alloc_tile_pool(name="rout", bufs=1)
            self_routing(nc, tc, rp, psR, psT, scores, bias_sb,
                         esel_sb, payhl_sb, iota_sb, ones_sb,
                         triu_sb, tok_rep, w_slots, tok_dram,
                         w_dram, cap_sched, s_core,
                         seams=[lambda t=t: emit_shared_tb(t)
                                for t in range(4)])
            rp.release()
            psBy.release()
            psB.release()
            stpB.release()
            shhp.release()
            psT.release()
            psR.release()
            shwp.release()
            cstp.release()

            # ---- 2. expert weight pools (allocated after routing so the
            #         routing-phase pools get the SBUF; e0/e1 loads issue
            #         now and overlap the first gather granules) ----
            wgp = tc.alloc_tile_pool(name="wgp", bufs=2)
            wup = tc.alloc_tile_pool(name="wup", bufs=2)
            wdp = tc.alloc_tile_pool(name="wdp", bufs=2)
            wg_t, wu_t, wd_t = {}, {}, {}

            def emit_weight_load(e):
                wge = wgp.tile([128, NH, INTER], BF16, tag="wge",
                               name=f"wge_{e}")
                nc.sync.dma_start(
                    wge[:], wg_d[e].rearrange("(j p) i -> p j i", p=128))
                wue = wup.tile([128, NH, INTER], BF16, tag="wue",
                               name=f"wue_{e}")
                nc.sync.dma_start(
                    wue[:], wu_d[e].rearrange("(j p) i -> p j i", p=128))
                wde = wdp.tile([128, 6, HID], BF16, tag="wde",
                               name=f"wde_{e}")
                for ci, (csz, coff) in enumerate(zip(IC_SIZES, IC_OFFS)):
                    nc.sync.dma_start(wde[0:csz, ci, :],
                                      wd_d[e, coff:coff + csz, :])
                wg_t[e] = wge
                wu_t[e] = wue
                wd_t[e] = wde

            emit_weight_load(0)
            emit_weight_load(1)

            # ---- 5. gather x^T (bf16) interleaved with expert compute ----
            xhp = tc.alloc_tile_pool(name="xhat", bufs=1)
            xhat = xhp.tile([128, NH, s_core], BF16, tag="xhall")
            psG = tc.alloc_tile_pool(name="psG", bufs=2, space="PSUM")
            iog = tc.alloc_tile_pool(name="iog", bufs=3)

            wp2 = tc.alloc_tile_pool(name="hp", bufs=1)
            stpE = tc.alloc_tile_pool(name="stE", bufs=4)
            psE = tc.alloc_tile_pool(name="psE", bufs=2, space="PSUM")
            psEy = tc.alloc_tile_pool(name="psEy", bufs=2, space="PSUM")

            # 256-row gather granules (tail may be 128)
            gchunks = []
            off = 0
            while off < s_core:
                rows = min(256, s_core - off)
                gchunks.append((off, rows))
                off += rows
            ngch = len(gchunks)

            gsems = []
            grows = []
            issued = [0]
            emitted = [0]          # granules fully transposed so far
            ysems = []
            stg_ring = []

            def issue_gather(c):
                goff, rows = gchunks[c]
                grow = iog.tile([128, 2, HID], FP32, tag="grow")
                gs_ = nc.alloc_semaphore(f"gx{c}")
                gp_ = nc.alloc_semaphore(f"gxp{c}")
                nc.gpsimd.dma_gather(
                    grow[:, 0:rows // 128, :], x_d[:],
                    tok_rep[:, goff // 16:(goff + rows) // 16],
                    rows, rows, HID, elem_step=HID,
                    prepare_only=True, sem=gs_,
                    queue_num=1).then_inc(gp_, 1)
                nc.gpsimd.wait_ge(gp_, 1)
                nc.gpsimd.trigger_dma(1, queue_num=1)
                gsems.append(gs_)
                grows.append(grow)
                issued[0] = c + 1

            def emit_chunk(c):
                # one critical per granule: issue ahead, then wait c's data
                with tc.tile_critical():
                    if c == 0:
                        issue_gather(0)
                    if issued[0] < ngch and issued[0] <= c + 1:
                        issue_gather(issued[0])
                    nc.gpsimd.wait_ge(gsems[c], 16)
                    nc.gpsimd.tensor_copy(grows[c][0:1, 0, 0:1],
                                          grows[c][0:1, 0, 0:1])
                goff, rows = gchunks[c]
                for s in range(rows // 128):
                    for a in range(2):
                        ptg = psG.tile([128, 512], FP32, tag="ptg")
                        for b in range(4):
                            j = 4 * a + b
                            nc.tensor.transpose(
                                ptg[:, 128 * b:128 * (b + 1)],
                                grows[c][:, s, 128 * j:128 * (j + 1)],
                                ident[:])
                        dst = xhat[:, 4 * a:4 * a + 4,
                                   goff + 128 * s:goff + 128 * (s + 1)]
                        src = ptg[:].rearrange("p (b q) -> p b q", b=4)
                        nc.vector.tensor_copy(dst, src)

            def ensure_chunks(nslots):
                # make sure slots [0, nslots) are gathered + transposed
                while emitted[0] < ngch and \
                        gchunks[emitted[0]][0] < nslots:
                    emit_chunk(emitted[0])
                    emitted[0] += 1

            CAPMAX = max(cap_sched)

            def emit_expert(e, lbase):
                cap = cap_sched[e]
                sizes = _blocks(cap)
                if e + 2 < EPC:
                    emit_weight_load(e + 2)
                wge, wue, wde = wg_t[e], wu_t[e], wd_t[e]
                He = [wp2.tile([128, CAPMAX], BF16, tag=f"he{ci}", bufs=2,
                               name=f"he{ci}_{e}") for ci in range(6)]
                for ci, (csz, coff) in enumerate(zip(IC_SIZES, IC_OFFS)):
                    boff = 0
                    for bsz in sizes:
                        bl = lbase + boff
                        pg = psE.tile([128, 512], FP32, tag="epg")
                        pu = psE.tile([128, 512], FP32, tag="epu")
                        for kk in range(NH):
                            nc.tensor.matmul(
                                pg[0:csz, 0:bsz],
                                wge[:, kk, coff:coff + csz],
                                xhat[:, kk, bl:bl + bsz],
                                start=(kk == 0), stop=(kk == NH - 1))
                        for kk in range(NH):
                            nc.tensor.matmul(
                                pu[0:csz, 0:bsz],
                                wue[:, kk, coff:coff + csz],
                                xhat[:, kk, bl:bl + bsz],
                                start=(kk == 0), stop=(kk == NH - 1))
                        hs = wp2.tile([128, 512], BF16, tag="hsil", bufs=3)
                        nc.scalar.activation(
                            hs[0:csz, 0:bsz], pg[0:csz, 0:bsz], ACTF.Silu)
                        nc.vector.tensor_tensor(
                            He[ci][0:csz, boff:boff + bsz],
                            hs[0:csz, 0:bsz],
                            pu[0:csz, 0:bsz], ALU.mult)
                        boff += bsz

                # 256-row scatter launches: 512 descriptors each, so two
                # fit in the 1024-desc SWDGE ring and descgen of launch
                # n+1 overlaps launch n's drain (512-row launches filled
                # the whole ring and DRAIN-stalled gpsimd 4-24us each)
                for yoff in range(0, cap, 256):
                    ysz = min(256, cap - yoff)
                    bl = lbase + yoff
                    if len(ysems) >= 4:
                        # stg ring anti-race: re-use of a stg buffer must
                        # wait for the scatter 4 launches back
                        old_sem, old_stg = stg_ring[len(ysems) - 4]
                        with tc.tile_critical():
                            nc.gpsimd.wait_ge(old_sem, 16)
                            nc.gpsimd.tensor_copy(old_stg[0:1, 0, 0:1],
                                                  old_stg[0:1, 0, 0:1])
                    stg = stpE.tile([128, 2, HID], OUT_DT, tag="ystg")
                    for sc_i in range(ysz // 128):
                        so = yoff + 128 * sc_i
                        col = (bl + 128 * sc_i) // 128
                        for nh2 in range(2):
                            py = psEy.tile([128, 512], FP32, tag="epy")
                            for ci, csz in enumerate(IC_SIZES):
                                nc.tensor.matmul(
                                    py[:],
                                    He[ci][0:csz, so:so + 128],
                                    wde[0:csz, ci,
                                        512 * nh2:512 * (nh2 + 1)],
                                    start=(ci == 0), stop=(ci == 5))
                            nc.scalar.mul(
                                stg[:, sc_i, 512 * nh2:512 * (nh2 + 1)],
                                py[:], w_slots[:, col:col + 1])
                    ysem = nc.alloc_semaphore(f"swdge_y{e}_{yoff}")
                    nc.gpsimd.dma_scatter_add(
                        out_part[:], stg[:, 0:ysz // 128, :],
                        tok_rep[:, bl // 16:(bl + ysz) // 16],
                        ysz, ysz, HID,
                        prepare_only=True, sem=ysem)
                    nc.gpsimd.trigger_dma(count=None)
                    ysems.append(ysem)
                    stg_ring.append((ysem, stg))

            lbase = 0
            for e in range(EPC):
                cap = cap_sched[e]
                ensure_chunks(lbase + cap)
                emit_expert(e, lbase)
                lbase += cap

            psEy.release()
            psE.release()
            stpE.release()
            wp2.release()
            iog.release()
            psG.release()
            xhp.release()
            wdp.release()
            wup.release()
            wgp.release()

            # ---- 6. drain scatters, chunked reduce-scatter, output ----
            with tc.tile_critical():
                for ys in ysems:
                    nc.gpsimd.wait_ge(ys, 16)
                d2sem = nc.alloc_semaphore("y_drain")
                nc.gpsimd.dma_start(
                    tokpay_sb[0:1, 0:1].bitcast(OUT_DT)[0:1, 0:1],
                    out_part[0:1, 0:1]).then_inc(d2sem, 16)
                nc.gpsimd.wait_ge(d2sem, 16)
            iop2 = tc.alloc_tile_pool(name="iop2", bufs=2)
            if COMB == "a2a":
                # mesh AllToAll (concurrent peer streams, unlike the ~20GB/s
                # RDH ring RS), then an on-chip 8-shard reduce. Core c
                # receives shard c of every rank's out_part: chunk j holds
                # rank j's partial rows for tokens [256c, 256c+256).
                a2a_out = dram.tile([T, HID], OUT_DT)
                nc.gpsimd.collective_compute(
                    "AllToAll", ALU.bypass,
                    replica_groups=[list(range(NC))],
                    ins=[out_part.opt()], outs=[a2a_out.opt()])
                shp = tc.alloc_tile_pool(name="shrd", bufs=4)
                TPC = T // NC           # 256 tokens per core
                for i in range(TPC // 128):
                    acc = iop2.tile([128, HID], FP32, tag="acc")
                    for j in range(NC):
                        sh_t = shp.tile([128, HID], OUT_DT, tag="sha")
                        nc.sync.dma_start(
                            sh_t[:],
                            a2a_out[TPC * j + 128 * i:
                                    TPC * j + 128 * (i + 1), :])
                        if j == 0:
                            nc.vector.tensor_copy(acc[:], sh_t[:])
                        else:
                            nc.vector.tensor_tensor(acc[:], acc[:],
                                                    sh_t[:], ALU.add)
                    nc.sync.dma_start(out_d[128 * i:128 * (i + 1), :],
                                      acc[:])
                shp.release()
            else:
                # RS in RS_CH chunks of [T/RS_CH, HID]; core c's chunk
                # shard is tokens [T/RS_CH*ch + T/NC/RS_CH*c, ...).
                # kernel() reassembles rows via unshard_out.
                TCH = T // RS_CH            # tokens per chunk
                OCH = TCH // NC             # out rows per chunk per core
                for ch in range(RS_CH):
                    nc.gpsimd.collective_compute(
                        "ReduceScatter", ALU.add,
                        replica_groups=[list(range(NC))],
                        ins=[out_part[TCH * ch:TCH * (ch + 1), :].opt()],
                        outs=[rs_out[OCH * ch:OCH * (ch + 1), :].opt()])
                for i in range(T // NC // 128):
                    ot2 = iop2.tile([128, HID], OUT_DT, tag="outld")
                    nc.sync.dma_start(ot2[:],
                                      rs_out[128 * i:128 * (i + 1), :])
                    of2 = iop2.tile([128, HID], FP32, tag="outf")
                    nc.scalar.copy(of2[:], ot2[:])
                    nc.sync.dma_start(out_d[128 * i:128 * (i + 1), :],
                                      of2[:])
            iop2.release()

    nc.compile()
    return nc


def self_routing(nc, tc, rp, psR, psT, scores, bias_sb, esel_sb, payhl_sb,
                 iota_sb, ones_sb, triu_sb, tok_rep, w_slots, tok_dram,
                 w_dram, cap_sched, s_core, seams=()):
    """Grouped top-k -> per-expert rank -> ON-CHIP slot table.

    Instead of scattering (t,k) payloads to HBM and reading back, the
    compact per-slot (token, weight) table is built on-chip: per expert
    position, a one-hot over rank (is_eq vs iota) is contracted with a
    (tok_hi, tok_lo, w) payload on the PE; tok = 128*hi + lo keeps token
    ids exact in bf16. `seams` are emission callbacks (shared-expert tb
    blocks) interleaved at points where routing's DVE chain has gaps."""
    V = nc.vector
    P = nc.gpsimd
    seams = list(seams)

    def seam():
        if seams:
            seams.pop(0)()

    # transient routing tiles, freed before the table-build pools
    rt = tc.alloc_tile_pool(name="rtmp", bufs=1)

    seam()
    sc_b = rt.tile([128, NTT, E], FP32, tag="scb")
    V.tensor_tensor(
        sc_b[:], scores[:],
        bias_sb[:].unsqueeze(1).to_broadcast([128, NTT, E]), ALU.add)

    scg = sc_b[:].rearrange("p t (g s) -> p t g s", g=NG, s=GS)
    m1 = rt.tile([128, NTT, NG], FP32, tag="m1")
    V.tensor_reduce(m1[:], scg, AXL.X, ALU.max)
    oh1 = rt.tile([128, NTT, NG, GS], FP32, tag="oh1")
    V.tensor_tensor(
        oh1[:], scg,
        m1[:].unsqueeze(3).to_broadcast([128, NTT, NG, GS]), ALU.is_ge)
    # masked second-max, in place over oh1
    V.scalar_tensor_tensor(oh1[:], oh1[:], NEG, scg, ALU.mult, ALU.add)
    m2 = rt.tile([128, NTT, NG], FP32, tag="m2")
    V.tensor_reduce(m2[:], oh1[:], AXL.X, ALU.max)
    gsc = rt.tile([128, NTT, NG], FP32, tag="gsc")
    V.tensor_tensor(gsc[:], m1[:], m2[:], ALU.add)

    seam()
    gmask = rt.tile([128, NTT, NG], FP32, tag="gmask")
    P.memset(gmask[:], 0.0)
    for g in range(TG):
        gm = rt.tile([128, NTT, 1], FP32, tag="gm")
        V.tensor_reduce(gm[:], gsc[:], AXL.X, ALU.max)
        ohg = rt.tile([128, NTT, NG], FP32, tag="ohg")
        V.tensor_tensor(ohg[:], gsc[:],
                        gm[:].to_broadcast([128, NTT, NG]), ALU.is_ge)
        V.tensor_tensor(gmask[:], gmask[:], ohg[:], ALU.add)
        V.scalar_tensor_tensor(gsc[:], ohg[:], NEG, gsc[:],
                               ALU.mult, ALU.add)

    # masked = sc where group selected else -1e30, built in place over sc_b
    masked = sc_b
    mview = scg
    V.tensor_tensor(
        mview, scg,
        gmask[:].unsqueeze(3).to_broadcast([128, NTT, NG, GS]), ALU.mult)
    gb = rt.tile([128, NTT, NG], FP32, tag="gb")
    V.tensor_scalar(gb[:], gmask[:], 1.0e30, -1.0e30, ALU.mult, ALU.add)
    V.tensor_tensor(
        mview, mview,
        gb[:].unsqueeze(3).to_broadcast([128, NTT, NG, GS]), ALU.add)

    seam()
    ohall = rt.tile([128, K, NTT, E], BF16, tag="ohall")
    for k in range(K):
        mk = rt.tile([128, NTT, 1], FP32, tag=f"mk{k}")
        V.tensor_reduce(mk[:], masked[:], AXL.X, ALU.max)
        ohk = ohall[:, k, :, :]
        V.tensor_tensor(ohk, masked[:],
                        mk[:].to_broadcast([128, NTT, E]), ALU.is_ge)
        V.scalar_tensor_tensor(masked[:], ohk, NEG, masked[:],
                               ALU.mult, ALU.add)

    seam()
    # rank matmul (bf16 0/1 inputs, fp32 psum), 4 token-tiles per PSUM bank
    msel = rt.tile([128, NTT, E], FP32, tag="msel")
    V.tensor_reduce(msel[:],
                    ohall[:].rearrange("p k t e -> p t e k"),
                    AXL.X, ALU.add)
    msel_bf = rt.tile([128, NTT, E], BF16, tag="mselbf")
    V.tensor_copy(msel_bf[:], msel[:])
    R = rt.tile([128, NTT, E], FP32, tag="R")
    for q in range(NTT // 4):
        pr = psR.tile([128, 4, E], FP32, tag="prq")
        for ii in range(4):
            i = 4 * q + ii
            n_mm = i + 1
            for mi in range(n_mm):
                lhsT = ones_sb[:] if mi < i else triu_sb[:]
                nc.tensor.matmul(pr[:, ii, :], lhsT, msel_bf[:, mi, :],
                                 start=(mi == 0), stop=(mi == n_mm - 1))
        V.tensor_copy(R[:, 4 * q:4 * q + 4, :], pr[:])

    # ---- W[t,e]: normalized expert weight x SCALE (0 if not selected) ----
    BIG = 1.0e9
    RmW = rp.tile([128, 2, NTT, E], FP32, tag="RmW")
    SW = rt.tile([128, NTT, E], FP32, tag="SW")
    V.tensor_tensor(SW[:], msel[:], scores[:], ALU.mult)
    den = rt.tile([128, NTT], FP32, tag="den")
    V.tensor_reduce(den[:], SW[:], AXL.X, ALU.add)
    rden = rt.tile([128, NTT], FP32, tag="rden")
    V.reciprocal(rden[:], den[:])
    rdenS = rt.tile([128, NTT], FP32, tag="rdenS")
    V.tensor_scalar_mul(rdenS[:], rden[:], SCALE)
    V.tensor_tensor(RmW[:, 1, :, :], SW[:],
                    rdenS[:].unsqueeze(2).to_broadcast([128, NTT, E]),
                    ALU.mult)

    # ---- Rm[t,e]: rank where selected, else BIG ----
    msel_u8 = rt.tile([128, NTT, E], mybir.dt.uint8, tag="mselu")
    V.tensor_copy(msel_u8[:], msel[:])
    P.memset(RmW[:, 0, :, :], BIG)
    V.copy_predicated(RmW[:, 0, :, :], msel_u8[:], R[:])
    rt.release()

    # ---- per expert position: one-hot over rank -> table matmul ----
    # fp16 keeps token ids (<= 2047) and 0/1 one-hots exact at full PE rate
    tab = rp.tile([2, s_core], FP32, tag="tab")
    selp = tc.alloc_tile_pool(name="selp", bufs=2)
    ohp = tc.alloc_tile_pool(name="ohp", bufs=2)
    lbase = 0
    for p in range(EPC):
        cap = cap_sched[p]
        tmp_pe = selp.tile([128, 2, NTT, E], FP32, tag="tmpPE")
        P.tensor_tensor(
            tmp_pe[:], RmW[:],
            esel_sb[:, p, :].unsqueeze(1).unsqueeze(1)
            .to_broadcast([128, 2, NTT, E]), ALU.mult)
        rmw_p = selp.tile([128, 2, NTT], FP32, tag="rmwp")
        V.tensor_reduce(rmw_p[:], tmp_pe[:], AXL.X, ALU.add)
        pay2 = selp.tile([128, NTT, 2], F16, tag="pay2")
        V.tensor_copy(pay2[:, :, 0], payhl_sb[:])
        V.tensor_copy(pay2[:, :, 1], rmw_p[:, 1, :])
        for B in range(cap // 128):
            rmb = selp.tile([128, NTT], F16, tag="rmb")
            V.tensor_scalar_add(rmb[:], rmw_p[:, 0, :], float(-128 * B))
            ohb = ohp.tile([128, NTT, 128], F16, tag="ohb")
            V.tensor_tensor(
                ohb[:], iota_sb[:],
                rmb[:].unsqueeze(2).to_broadcast([128, NTT, 128]),
                ALU.is_equal)
            pt = psT.tile([2, 128], FP32, tag="ptab")
            for i in range(NTT):
                nc.tensor.matmul(pt[:], pay2[:, i, :],
                                 ohb[:, i, :],
                                 start=(i == 0), stop=(i == NTT - 1))
            soff = lbase + 128 * B
            nc.scalar.copy(tab[:, soff:soff + 128], pt[:])
        lbase += cap
    ohp.release()
    selp.release()

    # unfilled slots give tok=0, w=0 (harmless)
    tok16 = rp.tile([1, s_core], I16, tag="tok16")
    V.tensor_copy(tok16[:], tab[0:1, :])
    nc.sync.dma_start(tok_dram[0:1, :], tok16[0:1, :])
    nc.sync.dma_start(w_dram[0:1, :], tab[1:2, :])
    nc.sync.dma_start(
        w_slots[:], w_dram[0, :].rearrange("(c p) -> p c", p=128))
    # one strided (descriptor-heavy) wrap read, then contiguous replicas
    nc.sync.dma_start(
        tok_rep[0:16, :],
        tok_dram[0, :].rearrange("(s q) -> q s", q=16))
    for b in range(1, 8):
        nc.sync.dma_start(tok_rep[16 * b:16 * (b + 1), :],
                          tok_rep[0:16, :])


def prepare(**inputs):
    import ml_dtypes

    x = np.ascontiguousarray(inputs["hidden_states"], dtype=np.float32)
    gate_w = np.ascontiguousarray(inputs["gate_w"], dtype=np.float32)
    score_bias = np.ascontiguousarray(inputs["score_bias"], dtype=np.float32)

    cnt = _host_routing_counts(x, gate_w, score_bias)
    caps = ((cnt + 16 + 127) // 128 * 128).astype(np.int64)
    bins, cap_sched = _placement(caps)
    s_core = int(sum(cap_sched))

    ident = np.eye(128, dtype=np.float32)
    ones128 = np.ones((128, 128), dtype=ml_dtypes.bfloat16)
    triu128 = np.triu(np.ones((128, 128), np.float32), 1).astype(
        ml_dtypes.bfloat16)
    tokpay = np.zeros((128, PAY_S), np.float32)
    for k in range(K):
        for i in range(NTT):
            tokpay[:, k * NTT + i] = np.arange(128) + 128 * i

    bf = ml_dtypes.bfloat16
    # token-id payload (fp16 holds ints <= 2048 exactly)
    payhl = np.zeros((128, NTT), np.float16)
    for i in range(NTT):
        payhl[:, i] = np.arange(128) + 128 * i
    iota_nt = np.tile(np.arange(128, dtype=np.float16)[None, None, :],
                      (128, NTT, 1))

    ish_c = ISH // NC
    in_maps = []
    for c in range(NC):
        esel = np.zeros((128, EPC, E), np.float32)
        for p, e in enumerate(bins[c]):
            esel[:, p, e] = 1.0
        perm = bins[c]
        in_maps.append({
            "x": x,
            "gwT": np.ascontiguousarray(gate_w.T.reshape(NH, 128, E)),
            "bias_b": np.ascontiguousarray(
                np.tile(score_bias, (128, 1))),
            "wg": np.ascontiguousarray(
                np.asarray(inputs["w_gate"], np.float32)[perm].astype(bf)),
            "wu": np.ascontiguousarray(
                np.asarray(inputs["w_up"], np.float32)[perm].astype(bf)),
            "wd": np.ascontiguousarray(
                np.asarray(inputs["w_down"], np.float32)[perm].astype(bf)),
            "shg": np.ascontiguousarray(
                np.asarray(inputs["sh_gate"],
                           np.float32)[:, c * ish_c:(c + 1) * ish_c]
                .astype(bf)),
            "shu": np.ascontiguousarray(
                np.asarray(inputs["sh_up"],
                           np.float32)[:, c * ish_c:(c + 1) * ish_c]
                .astype(bf)),
            "shd": np.ascontiguousarray(
                np.asarray(inputs["sh_down"],
                           np.float32)[c * ish_c:(c + 1) * ish_c, :]
                .astype(bf)),
            "ident": ident,
            "ones128": ones128,
            "triu128": triu128,
            "esel": esel,
            "payhl": np.ascontiguousarray(payhl),
            "iota_nt": np.ascontiguousarray(iota_nt),
            "tokpay": tokpay,
        })

    nc = build_graph(cap_sched, s_core)
    return nc, in_maps


def unshard_out(per_core):
    """Chunked-RS row mapping: core c's out rows [OCH*ch, OCH*(ch+1)) are
    global tokens [TCH*ch + OCH*c, ...)."""
    TCH = T // RS_CH
    OCH = TCH // NC
    out = np.empty((T, HID), np.float32)
    for c in range(NC):
        r = per_core[c]
        for ch in range(RS_CH):
            out[TCH * ch + OCH * c:TCH * ch + OCH * (c + 1)] = \
                r[OCH * ch:OCH * (ch + 1)]
    return out


def kernel(**inputs):
    nc, in_maps = prepare(**inputs)
    res = run_bass_kernel_spmd(nc, in_maps, core_ids=list(range(NC)))
    out = unshard_out([res.results[c]["out"] for c in range(NC)])
    return np.ascontiguousarray(out, dtype=np.float32)


if __name__ == "__main__":
    import reference
    inp = {k: np.asarray(v) for k, v in reference.setup_inputs().items()}
    out = kernel(**inp)
    print("out", out.shape, out.dtype)

